# revision 1
# baseline (speedup 1.0000x reference)
"""Deformable conv block (B=8, C=64, H=W=128, K=3) on 8 Trainium2 cores.

Data-parallel over batch: one sample per core. See design notes inline.
Self-contained: hardcodes all shapes; host packs weights, device runs a
bf16 mask-and-shift bilinear sampling pipeline with PE convs.
"""
import numpy as np
import ml_dtypes
from contextlib import ExitStack

import concourse.bass as bass
from concourse import bacc
import concourse.tile as tile
from concourse import mybir
from concourse.bass_utils import run_bass_kernel_spmd

bf16 = mybir.dt.bfloat16
f32 = mybir.dt.float32
f16 = mybir.dt.float16
Alu = mybir.AluOpType

def mkap(base_ap, extra_off, free_dims):
    """AP over base_ap's tensor: keep its partition dim, custom free dims."""
    p = list(base_ap.ap[0])
    return bass.AP(base_ap.tensor, base_ap.offset + extra_off, [p] + free_dims)


B, C, H, W = 8, 64, 128, 128
HW = H * W
NT = 9
XPF = 64 * 128 + 4  # xtc tile free size (pad 2 each side)


def _build():
    nc = bacc.Bacc()
    x_in = nc.dram_tensor("x", [C, HW], bf16, kind="ExternalInput")
    wA = nc.dram_tensor("wA", [5, 65, 128], bf16, kind="ExternalInput")
    wB = nc.dram_tensor("wB", [5, 65, 18], bf16, kind="ExternalInput")
    wM = nc.dram_tensor("wM", [NT, 64, 64], bf16, kind="ExternalInput")
    em = nc.dram_tensor("em", [128, 96], bf16, kind="ExternalInput")
    out = nc.dram_tensor("out", [C, HW], f32, kind="ExternalOutput")

    with tile.TileContext(nc, pool_alloc_mode="queue") as tc, ExitStack() as ctx:
        pw = ctx.enter_context(tc.tile_pool(name="pw", bufs=1))
        pxtc = ctx.enter_context(tc.tile_pool(name="pxtc", bufs=1))
        xtc = []
        for si in range(6):
            t_x = pxtc.tile([128, XPF], bf16, name=f"xtc{si}")
            xtc.append(t_x)
        pxp_cm = tc.tile_pool(name="pxp", bufs=1)
        pxp = pxp_cm.__enter__()

        xpad = pxp.tile([65, 16896], bf16, name="xpad")
        nc.vector.memset(xpad[0:64, 0:256], 0.0)
        nc.vector.memset(xpad[0:64, 16640:16896], 0.0)
        nc.vector.memset(xpad[64:65, :], 1.0)
        nc.gpsimd.dma_start(xpad[0:64, 256:16640], x_in[:])

        wAt = pw.tile([65, 5 * 128], bf16, name="wAt")
        nc.gpsimd.dma_start(
            wAt[:].rearrange("c (k o) -> c k o", k=5),
            wA[:].rearrange("k c o -> c k o"),
        )
        wBt = pw.tile([65, 5 * 18], bf16, name="wBt")
        nc.gpsimd.dma_start(
            wBt[:].rearrange("c (k o) -> c k o", k=5),
            wB[:].rearrange("k c o -> c k o"),
        )
        wMt = pw.tile([64, NT * 64], bf16, name="wMt")
        nc.gpsimd.dma_start(
            wMt[:].rearrange("c (t o) -> c t o", t=NT),
            wM[:].rearrange("t c o -> c t o"),
        )
        offt = pw.tile([128, 128 * 32], bf16, name="offt")
        emt = pw.tile([128, 96], bf16, name="emt")
        nc.gpsimd.dma_start(emt[:], em[:])

        # ---------------- shifted transposed-x copies ----------------
        with tc.tile_pool(name="pxty", bufs=2) as pxty:
            for si in range(6):
                s = si - 2
                xty = pxty.tile([128, 128, 64], bf16, tag="xty", name="xty")
                nc.scalar.dma_start_transpose(
                    xty[:], xpad[0:64, 256 + s : 256 + s + HW]
                )
                dst = mkap(xtc[si][:], 2, [[128, 64], [1, 128]])
                nc.scalar.copy(out=dst, in_=xty[:].rearrange("x y c -> x c y"))
                nc.vector.memset(xtc[si][:, 0:2], 0.0)
                nc.vector.memset(xtc[si][:, XPF - 2 : XPF], 0.0)

        # ---------------- offset conv ----------------
        with tc.tile_pool(name="poff", bufs=2) as poff, tc.tile_pool(
            name="psoff", bufs=2, space="PSUM"
        ) as psoff:
            offacc = poff.tile([32, HW], bf16, name="offacc", bufs=1)
            for q in range(32):
                pA = psoff.tile([128, 512], f32, tag="pA", name="pA")
                pB = psoff.tile([32, 512], f32, tag="pB", name="pB")
                for ky in range(5):
                    rhs = xpad[:, 256 + q * 512 + (ky - 2) * 128 :][:, 0:512]
                    nc.tensor.matmul(
                        pA[:], wAt[:, ky * 128 : ky * 128 + 128], rhs,
                        start=(ky == 0), stop=(ky == 4),
                    )
                for ky in range(5):
                    rhs = xpad[:, 256 + q * 512 + (ky - 2) * 128 :][:, 0:512]
                    nc.tensor.matmul(
                        pB[0:18, :], wBt[:, ky * 18 : ky * 18 + 18], rhs,
                        start=(ky == 0), stop=(ky == 4),
                    )
                stA = poff.tile([128, 512], bf16, tag="stA", name="stA")
                stB = poff.tile([32, 512], bf16, tag="stB", name="stB")
                nc.scalar.copy(stA[:], pA[:])
                nc.scalar.copy(stB[0:18, :], pB[0:18, :])
                # init: kx=2 block (includes bias via ones-row)
                nc.gpsimd.dma_start(
                    offacc[:, q * 512 : (q + 1) * 512], stA[64:96, :]
                )
                # accumulate the other kx blocks with x-windows (rows of 128)
                oav = offacc[:].rearrange("d (y x) -> d y x", x=W)[
                    :, 4 * q : 4 * q + 4, :
                ]
                sAv = stA[:].rearrange("d (y x) -> d y x", x=W)
                sBv = stB[:].rearrange("d (y x) -> d y x", x=W)
                for base, co in [(0, -2), (32, -1), (96, 1), (-1, 2)]:
                    xs, xe = max(0, -co), min(W, W - co)
                    if base < 0:
                        srcw = sBv[0:18, :, xs + co : xe + co]
                    else:
                        srcw = sAv[base : base + 18, :, xs + co : xe + co]
                    nc.gpsimd.dma_start(
                        out=oav[0:18, :, xs:xe], in_=srcw, accum_op=Alu.add
                    )
            nc.vector.tensor_scalar(
                out=offacc[0:18, :], in0=offacc[0:18, :],
                scalar1=1.0, scalar2=-1.0, op0=Alu.min, op1=Alu.max,
            )
            nc.scalar.dma_start_transpose(
                offt[:].rearrange("x (y d) -> x y d", d=32), offacc[:]
            )

        pxp_cm.__exit__(None, None, None)  # free xpad

        # ---------------- mask / weight maps ----------------
        pgg = ctx.enter_context(tc.tile_pool(name="pgg", bufs=1))
        gg = pgg.tile([128, 81 * 128], bf16, name="gg")
        with tc.tile_pool(name="pg", bufs=1) as pg:
            mneg = pg.tile([128, 128 * 32], bf16, name="mneg")
            nc.vector.tensor_scalar(
                out=mneg[:], in0=offt[:], scalar1=0.0, scalar2=None, op0=Alu.is_lt
            )
            fr = pg.tile([128, 128 * 32], bf16, name="fr")
            nc.vector.tensor_tensor(out=fr[:], in0=offt[:], in1=mneg[:], op=Alu.add)
            omf = pg.tile([128, 128 * 32], bf16, name="omf")
            nc.vector.tensor_scalar(
                out=omf[:], in0=fr[:], scalar1=-1.0, scalar2=1.0,
                op0=Alu.mult, op1=Alu.add,
            )
            g = pg.tile([128, 3 * 128 * 32], bf16, name="g")
            t1 = pg.tile([128, 128 * 32], bf16, name="t1")
            g0 = g[:, 0 : 128 * 32]
            g1 = g[:, 128 * 32 : 2 * 128 * 32]
            g2_ = g[:, 2 * 128 * 32 : 3 * 128 * 32]
            nc.vector.tensor_tensor(out=g0, in0=mneg[:], in1=omf[:], op=Alu.mult)
            nc.vector.tensor_tensor(out=t1[:], in0=mneg[:], in1=fr[:], op=Alu.mult)
            nc.vector.tensor_tensor(out=g2_, in0=fr[:], in1=t1[:], op=Alu.subtract)
            nc.vector.tensor_tensor(out=g1, in0=t1[:], in1=omf[:], op=Alu.add)
            nc.vector.tensor_tensor(out=g1, in0=g1, in1=g0, op=Alu.subtract)

            gv = g[:].rearrange("x (r y d) -> x r y d", r=3, d=32)
            # y-bound masks: zero gy (cols 2*(3ti+tj), tj=0..2) at edge rows
            for (ti, ry, ys, ye) in [
                (0, 0, 0, 2), (0, 1, 0, 1), (1, 0, 0, 1),
                (1, 2, 127, 128), (2, 1, 127, 128), (2, 2, 126, 128),
            ]:
                for tj in range(3):
                    c0 = 6 * ti + 2 * tj
                    nc.vector.memset(gv[:, ry, ys:ye, c0 : c0 + 1], 0.0)
            # x-bound masks: full-width multiply; em2 is 1.0 except dx cols
            for rx in range(3):
                blk = gv[:, rx, :, :]
                em_ap = mkap(emt[:], rx * 32, [[0, 128], [1, 32]])
                nc.vector.tensor_tensor(out=blk, in0=blk, in1=em_ap, op=Alu.mult)

            # gg[x, (ti tj ry rx y)] = gy * gx
            ggv = gg[:].rearrange(
                "x (ti tj ry rx y) -> x ti tj ry rx y", ti=3, tj=3, ry=3, rx=3
            )
            gt = g[:].tensor
            goff = g[:].offset
            for ry in range(3):
                for ti in range(3):
                    gy_ap = mkap(
                        g[:], ry * 4096 + 6 * ti,
                        [[2, 3], [0, 3], [32, 128]],
                    )
                    gx_ap = mkap(
                        g[:], 6 * ti + 1,
                        [[2, 3], [4096, 3], [32, 128]],
                    )
                    nc.vector.tensor_tensor(
                        out=ggv[:, ti, :, ry, :, :], in0=gy_ap, in1=gx_ap,
                        op=Alu.mult,
                    )

        # ---------------- sampling + per-tap finalize ----------------
        pout = ctx.enter_context(tc.tile_pool(name="pout", bufs=1))
        outacc = pout.tile([128, 8192], bf16, name="outacc")
        nc.gpsimd.memset(outacc[:], 0.0)

        pacc = ctx.enter_context(tc.tile_pool(name="pacc", bufs=1))
        ptmp = ctx.enter_context(tc.tile_pool(name="ptmp", bufs=1))
        pfin = ctx.enter_context(tc.tile_pool(name="pfin", bufs=1))
        psm = ctx.enter_context(tc.tile_pool(name="psm", bufs=2, space="PSUM"))

        aycz = pfin.tile([128, 32 * 128], bf16, tag="ayc", name="aycz")
        nc.vector.memset(aycz[:], 0.0)
        for t in range(NT):
            ti, tj = t // 3, t % 3
            acc = pacc.tile([128, 8192], bf16, tag="acc", name="acc")
            first = True
            for ry in range(3):
                ro = ti - 2 + ry
                for rx in range(3):
                    co = tj - 2 + rx
                    m = t * 9 + ry * 3 + rx
                    xs_t = xtc[co + 2][:]
                    in0 = mkap(xs_t, 2 + ro, [[128, 64], [1, 128]])
                    in1 = mkap(gg[:], m * 128, [[0, 64], [1, 128]])
                    if first:
                        o_ap = mkap(acc[:], 0, [[128, 64], [1, 128]])
                        nc.vector.tensor_tensor(
                            out=o_ap, in0=in0, in1=in1, op=Alu.mult
                        )
                        first = False
                    else:
                        tmp = ptmp.tile([128, 8192], bf16, tag="tmp", name="tmp")
                        nc.vector.tensor_tensor(
                            out=tmp[:].rearrange("x (c y) -> x c y", y=128),
                            in0=in0, in1=in1, op=Alu.mult,
                        )
                        nc.vector.tensor_tensor(
                            out=acc[:], in0=acc[:], in1=tmp[:], op=Alu.add
                        )

            # finalize tap: 4 y-quarters of 32 rows
            for q in range(4):
                ayc = pfin.tile([128, 32 * 128], bf16, tag="ayc", name="ayc")
                srcp = mkap(acc[:], 32 * q, [[1, 32], [128, 64]])
                nc.scalar.copy(
                    out=mkap(ayc[:], 0, [[128, 32], [1, 64]]), in_=srcp
                )
                sch = pfin.tile([64, 32, 128], bf16, tag="sch", name="sch")
                nc.scalar.dma_start_transpose(sch[:], ayc[:])
                for hh in range(2):
                    pm = psm.tile([64, 2048], f32, tag="pm", name="pm")
                    for j in range(4):
                        nc.tensor.matmul(
                            pm[:, j * 512 : (j + 1) * 512],
                            wMt[:, t * 64 : (t + 1) * 64],
                            sch[:, hh * 16 + 4 * j : hh * 16 + 4 * j + 4, :]
                            .rearrange("c a x -> c (a x)"),
                            start=True, stop=True,
                        )
                    tpd = pfin.tile([64, 2048], bf16, tag="tpd", name="tpd")
                    nc.scalar.copy(tpd[:], pm[:])
                    p0 = q * 4096 + hh * 2048
                    half, off = divmod(p0, 8192)
                    oslice = outacc[64 * half : 64 * half + 64, off : off + 2048]
                    nc.gpsimd.dma_start(
                        out=oslice, in_=tpd[:], accum_op=Alu.add
                    )

        nc.gpsimd.dma_start(out[:, 0:8192], outacc[0:64, :])
        nc.gpsimd.dma_start(out[:, 8192:16384], outacc[64:128, :])

    nc.compile()
    return nc


_NC = None


def _get_nc():
    global _NC
    if _NC is None:
        _NC = _build()
    return _NC


def kernel(x, weights, offset_w, offset_b):
    x = np.asarray(x, dtype=np.float32)
    weights = np.asarray(weights, dtype=np.float32)
    offset_w = np.asarray(offset_w, dtype=np.float32)
    offset_b = np.asarray(offset_b, dtype=np.float32)

    wA = np.zeros((5, 65, 128), np.float32)
    for kx in range(4):
        wA[:, 0:64, kx * 32 : kx * 32 + 18] = offset_w[:, :, :, kx].transpose(2, 1, 0)
    wA[2, 64, 64 : 64 + 18] = offset_b
    wB = np.zeros((5, 65, 18), np.float32)
    wB[:, 0:64, :] = offset_w[:, :, :, 4].transpose(2, 1, 0)
    wM = weights.reshape(C, C, 9).transpose(2, 1, 0).copy()
    em = np.ones((128, 96), np.float32)
    xs_ = np.arange(128)
    for rx in range(3):
        for ti in range(3):
            for tj in range(3):
                co = tj - 2 + rx
                em[:, rx * 32 + 2 * (3 * ti + tj) + 1] = (
                    (xs_ + co >= 0) & (xs_ + co < 128)
                )

    cast = lambda a: np.ascontiguousarray(a).astype(ml_dtypes.bfloat16)
    in_maps = [
        {
            "x": cast(x[b].reshape(C, HW)),
            "wA": cast(wA),
            "wB": cast(wB),
            "wM": cast(wM),
            "em": cast(em),
        }
        for b in range(B)
    ]
    nc = _get_nc()
    # Per-sample sequential execution. The 8-core shard_map path triggers an
    # engine hang (NRT_EXEC_UNIT_UNRECOVERABLE) that wedges the device for
    # the rest of the process, so run one core at a time.
    outs = []
    for b in range(B):
        r1 = run_bass_kernel_spmd(nc, [in_maps[b]], [0])
        outs.append(np.asarray(r1.results[0]["out"]))
    return np.stack([o.reshape(C, H, W) for o in outs]).astype(np.float32)

